# revision 13
# baseline (speedup 1.0000x reference)
"""PhysicsAttention kernel for 8 TRN2 cores.

Sharding: core c handles batch b=c//2, N-half c%2 -> x shard [8192, 256].
Pass 1 computes unnormalized exp(logits) (eT, [64, 8192] per core) and the
PSUM-accumulated pooled sums [tokens | z] = e.T @ [x | 1]; a pair AllReduce
(cores 2b, 2b+1) combines the N-halves. The small M=64-token attention is
replicated per core; pass 2 un-pools out = eT.T @ (token_out / z).
"""

import math
import sys
from contextlib import ExitStack

import numpy as np

try:
    import concourse.bass as bass
except ImportError:
    sys.path.insert(0, "/opt/trn_rl_repo")
    import concourse.bass as bass

import concourse.tile as tile
from concourse import bacc, mybir
from concourse.bass_utils import run_bass_kernel_spmd
from concourse.masks import make_identity

B, N, D = 4, 16384, 256
M, H, HD = 64, 8, 32
E = 3 * D  # 768
NSH = N // 2  # 8192 rows per core
P = 128
NT = NSH // P  # 64 tiles of 128 rows
GRP = 4  # tiles per DMA group
NG = NT // GRP  # 16 groups
ATTN_SCALE = 1.0 / math.sqrt(HD)

F32 = mybir.dt.float32
F32R = mybir.dt.float32r

# f32r (1 cyc/row at free>=256) toggles per matmul site; verified vs fp32.
R_LOGITS = True
R_POOL = True
R_V = True
R_WO = True
R_UNPOOL = True

AF = mybir.ActivationFunctionType


def _rr(ap, on=True):
    return ap.bitcast(F32R) if on else ap


def build_program():
    nc = bacc.Bacc(
        "TRN2", target_bir_lowering=False, debug=False, num_devices=8
    )

    x_d = nc.dram_tensor("x", [NSH, D], F32, kind="ExternalInput")
    ws_d = nc.dram_tensor("Ws", [M, D], F32, kind="ExternalInput")
    bs_d = nc.dram_tensor("bs", [M], F32, kind="ExternalInput")
    wqkv_d = nc.dram_tensor("Wqkv", [E, D], F32, kind="ExternalInput")
    wo_d = nc.dram_tensor("Wo", [D, D], F32, kind="ExternalInput")
    bo_d = nc.dram_tensor("bo", [D], F32, kind="ExternalInput")
    out_d = nc.dram_tensor("out", [NSH, D], F32, kind="ExternalOutput")

    xv = x_d.ap().rearrange("(g j p) d -> g p j d", j=GRP, p=P)
    outv = out_d.ap().rearrange("(g j p) d -> g p j d", j=GRP, p=P)

    with tile.TileContext(nc) as tc, ExitStack() as ctx:
        consts = ctx.enter_context(tc.tile_pool(name="consts", bufs=1))
        dram = ctx.enter_context(
            tc.tile_pool(name="dram", bufs=1, space=bass.MemorySpace.DRAM)
        )

        ident = consts.tile([P, P], F32)
        make_identity(nc, ident)

        # ---- weight staging + transposes (one-time) ----
        ws_raw = consts.tile([M, D], F32)
        nc.sync.dma_start(ws_raw[:], ws_d[:])
        wq_raw = consts.tile([P, 6, D], F32)
        nc.sync.dma_start(
            wq_raw[:], wqkv_d.ap().rearrange("(r p) d -> p r d", r=6, p=P)
        )
        wo_raw = consts.tile([P, 2, D], F32)
        nc.sync.dma_start(
            wo_raw[:], wo_d.ap().rearrange("(r p) d -> p r d", r=2, p=P)
        )
        bsT = consts.tile([M, 1], F32)
        nc.sync.dma_start(bsT[:], bs_d.ap().rearrange("(p o) -> p o", o=1))
        bo_b = consts.tile([M, D], F32)
        bo_ap = bo_d.ap()
        nc.sync.dma_start(
            bo_b[:],
            bass.AP(tensor=bo_ap.tensor, offset=bo_ap.offset,
                    ap=[[0, M], list(bo_ap.ap[0])]),
        )

        wsT = consts.tile([P, 2, M], F32)   # [d-in-chunk, c, m]
        wqT = consts.tile([P, 2, E], F32)   # [d-in-chunk, c, e]
        woT = consts.tile([M, 4, D], F32)   # [e'-in-chunk(64), c4, d]
        with tc.tile_pool(name="setup_ps", bufs=2,
                          space=bass.MemorySpace.PSUM) as sps:
            for c in range(2):
                t_ps = sps.tile([P, M], F32, name=f"wsT_ps{c}", tag="w")
                nc.tensor.transpose(t_ps[:], ws_raw[:, c * P:(c + 1) * P],
                                    ident[:M, :M])
                nc.any.tensor_copy(_rr(wsT[:, c, :], R_LOGITS), t_ps[:])
            for r in range(6):
                for c in range(2):
                    t_ps = sps.tile([P, P], F32, name=f"wqT_ps{r}_{c}", tag="w")
                    nc.tensor.transpose(t_ps[:], wq_raw[:, r, c * P:(c + 1) * P],
                                        ident[:])
                    nc.any.tensor_copy(_rr(wqT[:, c, r * P:(r + 1) * P], R_V), t_ps[:])
            for r in range(2):
                for c4 in range(4):
                    t_ps = sps.tile([M, P], F32, name=f"woT_ps{r}_{c4}", tag="w")
                    nc.tensor.transpose(t_ps[:],
                                        wo_raw[:, r, c4 * M:(c4 + 1) * M],
                                        ident[:])
                    nc.any.tensor_copy(_rr(woT[:, c4, r * P:(r + 1) * P], R_WO), t_ps[:])

        # ---- persistent buffers ----
        eT_all = consts.tile([M, NSH], F32)   # exp(logits).T, unnormalized
        tok_sb = consts.tile([M, D + 2], F32)
        tok_full = consts.tile([M, D + 2], F32)
        t_sb = consts.tile([M, D], F32)       # token_out / z, [m, d]
        ar_in = dram.tile([M, D + 2], F32)
        ar_out = dram.tile([M, D + 2], F32)

        # ones source for the 2 appended pooling columns (f32r matmuls
        # need even innermost free counts, and Memset cannot write f32r)
        ones_sb = consts.tile([P, 2 * GRP], F32)
        nc.vector.memset(ones_sb[:], 1.0)
        oap = ones_sb[:]
        ones_src = bass.AP(tensor=oap.tensor, offset=oap.offset,
                           ap=[list(oap.ap[0]), [2, GRP], [1, 2]])

        # ---- pass 1 ----
        with (
            tc.tile_pool(name="xa_pool", bufs=3) as xa_pool,
            tc.tile_pool(name="xT_sb_pool", bufs=2) as xT_sb_pool,
            tc.tile_pool(name="e_sb_pool", bufs=2) as e_sb_pool,
            tc.tile_pool(name="xT_ps_pool", bufs=2,
                         space=bass.MemorySpace.PSUM) as xT_ps_pool,
            tc.tile_pool(name="lg_ps_pool", bufs=2,
                         space=bass.MemorySpace.PSUM) as lg_ps_pool,
            tc.tile_pool(name="e_ps_pool", bufs=2,
                         space=bass.MemorySpace.PSUM) as e_ps_pool,
            tc.tile_pool(name="tok_ps_pool", bufs=1,
                         space=bass.MemorySpace.PSUM) as tok_ps_pool,
        ):
            tok_ps = tok_ps_pool.tile([M, D + 2], F32)
            for g in range(NG):
                xa = xa_pool.tile([P, GRP, D + 2], F32)
                nc.any.tensor_copy(_rr(xa[:, :, D:D + 2], R_POOL), ones_src)
                nc.sync.dma_start(_rr(xa[:, :, 0:D], R_POOL),
                                  _rr(xv[g], R_POOL))
                for pr in range(GRP // 2):  # pairs of tiles
                    i0 = g * GRP + pr * 2          # first tile index of pair
                    # transpose 2 tiles x 2 d-chunks into one PSUM bank:
                    # cols [c*256 + jj*128, +128) = x_tile(i0+jj)[:, cP:cP+P].T
                    xT_ps = xT_ps_pool.tile([P, 512], F32)
                    for c in range(2):
                        for jj in range(2):
                            nc.tensor.transpose(
                                xT_ps[:, c * 256 + jj * P: c * 256 + (jj + 1) * P],
                                xa[:, pr * 2 + jj, c * P:(c + 1) * P],
                                ident[:],
                            )
                    xT_sb = xT_sb_pool.tile([P, 512], F32)
                    nc.any.tensor_copy(_rr(xT_sb[:], R_LOGITS), xT_ps[:])
                    # logitsT [m, n-pair] accumulated over 2 d-chunks
                    lg_ps = lg_ps_pool.tile([M, 256], F32)
                    for c in range(2):
                        nc.tensor.matmul(
                            lg_ps[:],
                            _rr(wsT[:, c, :], R_LOGITS),
                            _rr(xT_sb[:, c * 256:(c + 1) * 256], R_LOGITS),
                            start=(c == 0), stop=(c == 1),
                        )
                    # eT = exp(logitsT + bs) straight into persistent store
                    nc.scalar.activation(
                        _rr(eT_all[:, i0 * P:(i0 + 2) * P], R_UNPOOL),
                        lg_ps[:], AF.Exp, bias=bsT[:], scale=1.0,
                    )
                    # e [n, m] via PE transpose of the eT slices
                    e_ps = e_ps_pool.tile([P, 2 * M], F32)
                    for jj in range(2):
                        nc.tensor.transpose(
                            e_ps[:, jj * M:(jj + 1) * M],
                            eT_all[:, (i0 + jj) * P:(i0 + jj + 1) * P],
                            ident[:M, :M],
                        )
                    e_sb = e_sb_pool.tile([P, 2 * M], F32)
                    nc.any.tensor_copy(_rr(e_sb[:], R_POOL), e_ps[:])
                    # pooled sums [tokens | z] += e.T @ [x | 1]
                    for jj in range(2):
                        it = i0 + jj
                        nc.tensor.matmul(
                            tok_ps[:],
                            _rr(e_sb[:, jj * M:(jj + 1) * M], R_POOL),
                            _rr(xa[:, pr * 2 + jj, :], R_POOL),
                            start=(it == 0), stop=(it == NT - 1),
                        )
            nc.any.tensor_copy(tok_sb[:], tok_ps[:])

        # ---- pair AllReduce of [tokens | z] ----
        nc.gpsimd.dma_start(ar_in[:], tok_sb[:])
        nc.gpsimd.collective_compute(
            "AllReduce",
            mybir.AluOpType.add,
            replica_groups=[[0, 1], [2, 3], [4, 5], [6, 7]],
            ins=[ar_in.opt()],
            outs=[ar_out.opt()],
        )
        nc.gpsimd.dma_start(tok_full[:], ar_out[:])

        # ---- attention over M=64 pooled tokens (replicated) ----
        with (
            tc.tile_pool(name="at_sb", bufs=1) as at_sb,
            tc.tile_pool(name="at_ps", bufs=4,
                         space=bass.MemorySpace.PSUM) as at_ps,
            tc.tile_pool(name="toT_ps_pool", bufs=1,
                         space=bass.MemorySpace.PSUM) as toT_ps_pool,
        ):
            rz = at_sb.tile([M, 1], F32)
            nc.vector.reciprocal(rz[:], tok_full[:, D:D + 1])
            tn = at_sb.tile([M, D], F32)
            nc.vector.tensor_scalar_mul(tn[:], tok_full[:, 0:D], rz[:])

            # tokT [d, m]
            tokT = at_sb.tile([P, 2, M], F32)
            tokT_ps = at_ps.tile([P, 2 * M], F32, tag="at")
            for c in range(2):
                nc.tensor.transpose(tokT_ps[:, c * M:(c + 1) * M],
                                    tn[:, c * P:(c + 1) * P], ident[:M, :M])
            nc.any.tensor_copy(
                _rr(tokT[:].rearrange("p c m -> p (c m)"), R_V), tokT_ps[:]
            )

            # qT, kT [64, 4, m]: chunk j = heads 2j,2j+1 ; v [m, 256]
            qT = at_sb.tile([M, 4, M], F32)
            kT = at_sb.tile([M, 4, M], F32)
            for which, dst in ((0, qT), (1, kT)):
                qk_ps = at_ps.tile([M, 4 * M], F32, name=f"qk_ps{which}", tag="at")
                for j in range(4):
                    off = which * D + j * M
                    for c in range(2):
                        nc.tensor.matmul(
                            qk_ps[:, j * M:(j + 1) * M],
                            wqT[:, c, off:off + M],
                            tokT[:, c, :],
                            start=(c == 0), stop=(c == 1),
                        )
                nc.any.tensor_copy(
                    dst[:].rearrange("p c m -> p (c m)"), qk_ps[:]
                )
            v_ps = at_ps.tile([M, D], F32, tag="at")
            for c in range(2):
                nc.tensor.matmul(
                    v_ps[:],
                    _rr(tokT[:, c, :], R_V),
                    _rr(wqT[:, c, 2 * D:3 * D], R_V),
                    start=(c == 0), stop=(c == 1),
                )
            v_sb = at_sb.tile([M, D], F32)
            nc.any.tensor_copy(v_sb[:], v_ps[:])

            # per-head attention; toT [64, 4, m] (chunk j = heads 2j,2j+1)
            toT_sb = at_sb.tile([M, 4, M], F32)
            toT_ps = toT_ps_pool.tile([M, 4 * M], F32, name="toT_ps")
            for h in range(H):
                chunk = h // 2
                row = (h % 2) * 32
                s_ps = at_ps.tile([M, M], F32, name=f"s_ps{h}", tag="at")
                nc.tensor.matmul(
                    s_ps[:],
                    qT[row:row + 32, chunk, :],
                    kT[row:row + 32, chunk, :],
                )
                mx = at_sb.tile([M, 1], F32, name=f"mx{h}")
                nc.vector.tensor_reduce(
                    mx[:], s_ps[:], axis=mybir.AxisListType.X,
                    op=mybir.AluOpType.max,
                )
                nc.vector.tensor_scalar_mul(mx[:], mx[:], -ATTN_SCALE)
                a_sb = at_sb.tile([M, M], F32, name=f"a_sb{h}")
                nc.scalar.activation(a_sb[:], s_ps[:], AF.Exp,
                                     bias=mx[:], scale=ATTN_SCALE)
                sm = at_sb.tile([M, 1], F32, name=f"sm{h}")
                nc.vector.tensor_reduce(
                    sm[:], a_sb[:], axis=mybir.AxisListType.X,
                    op=mybir.AluOpType.add,
                )
                nc.vector.reciprocal(sm[:], sm[:])
                nc.vector.tensor_scalar_mul(a_sb[:], a_sb[:], sm[:])
                aT_ps = at_ps.tile([M, M], F32, name=f"aT_ps{h}", tag="at")
                nc.tensor.transpose(aT_ps[:], a_sb[:], ident[:M, :M])
                aT_sb = at_sb.tile([M, M], F32, name=f"aT_sb{h}")
                nc.any.tensor_copy(aT_sb[:], aT_ps[:])
                # toT_h [hd, m] = v_h.T @ A.T
                nc.tensor.matmul(
                    toT_ps[row:row + 32, chunk * M:(chunk + 1) * M],
                    v_sb[:, h * 32:(h + 1) * 32],
                    aT_sb[:],
                )
            nc.any.tensor_copy(
                _rr(toT_sb[:].rearrange("p c m -> p (c m)"), R_WO), toT_ps[:]
            )

            # t = (toT.T @ WoT + bo) * rz
            t_ps = at_ps.tile([M, D], F32, name="t_ps", tag="at")
            for c4 in range(4):
                nc.tensor.matmul(
                    t_ps[:],
                    _rr(toT_sb[:, c4, :], R_WO),
                    _rr(woT[:, c4, :], R_WO),
                    start=(c4 == 0), stop=(c4 == 3),
                )
            nc.vector.tensor_add(_rr(t_sb[:], R_UNPOOL), t_ps[:], bo_b[:])
            nc.vector.tensor_scalar_mul(_rr(t_sb[:], R_UNPOOL), t_sb[:], rz[:])

        # ---- pass 2: out = eT.T @ t ----
        with (
            tc.tile_pool(name="o_sb_pool", bufs=3) as o_sb_pool,
            tc.tile_pool(name="o_ps_pool", bufs=4,
                         space=bass.MemorySpace.PSUM) as o_ps_pool,
        ):
            for g in range(NG):
                o_sb = o_sb_pool.tile([P, GRP, D], F32)
                for j in range(GRP):
                    it = g * GRP + j
                    o_ps = o_ps_pool.tile([P, D], F32)
                    nc.tensor.matmul(
                        o_ps[:],
                        _rr(eT_all[:, it * P:(it + 1) * P], R_UNPOOL),
                        _rr(t_sb[:], R_UNPOOL),
                    )
                    nc.any.tensor_copy(o_sb[:, j, :], o_ps[:])
                nc.sync.dma_start(outv[g], o_sb[:])

    nc.compile()
    return nc


_cached = None


def kernel(**inputs: np.ndarray) -> np.ndarray:
    global _cached
    if _cached is None:
        _cached = build_program()
    nc = _cached

    x = np.ascontiguousarray(inputs["x"], dtype=np.float32)
    shared = {
        "Ws": np.ascontiguousarray(inputs["Ws"], dtype=np.float32),
        "bs": np.ascontiguousarray(inputs["bs"], dtype=np.float32),
        "Wqkv": np.ascontiguousarray(inputs["Wqkv"], dtype=np.float32),
        "Wo": np.ascontiguousarray(inputs["Wo"], dtype=np.float32),
        "bo": np.ascontiguousarray(inputs["bo"], dtype=np.float32),
    }
    in_maps = []
    for c in range(8):
        b, half = c // 2, c % 2
        in_maps.append(
            {"x": np.ascontiguousarray(x[b, half * NSH:(half + 1) * NSH, :]),
             **shared}
        )
    res = run_bass_kernel_spmd(nc, in_maps, list(range(8)))
    out = np.empty((B, N, D), dtype=np.float32)
    for c in range(8):
        b, half = c // 2, c % 2
        out[b, half * NSH:(half + 1) * NSH, :] = res.results[c]["out"]
    return out


if __name__ == "__main__":
    rng = np.random.default_rng(0)
    ins = {
        "x": rng.standard_normal((B, N, D), dtype=np.float32),
        "Ws": rng.standard_normal((M, D), dtype=np.float32) / 16,
        "bs": np.zeros((M,), np.float32),
        "Wqkv": rng.standard_normal((E, D), dtype=np.float32) / 16,
        "Wo": rng.standard_normal((D, D), dtype=np.float32) / 16,
        "bo": np.zeros((D,), np.float32),
    }
    o = kernel(**ins)
    print(o.shape, o.dtype, float(np.abs(o).mean()))


# revision 14
# speedup vs baseline: 53.4446x; 53.4446x over previous
"""PhysicsAttention kernel for 8 TRN2 cores.

Sharding: core c handles batch b=c//2, N-half c%2 -> x shard [8192, 256].
Pass 1 computes unnormalized exp(logits) (eT, [64, 8192] per core) and the
PSUM-accumulated pooled sums [tokens | z] = e.T @ [x | 1]; a pair AllReduce
(cores 2b, 2b+1) combines the N-halves. The small M=64-token attention is
replicated per core; pass 2 un-pools out = eT.T @ (token_out / z).
"""

import math
import sys
from contextlib import ExitStack

import numpy as np

try:
    import concourse.bass as bass
except ImportError:
    sys.path.insert(0, "/opt/trn_rl_repo")
    import concourse.bass as bass

import concourse.tile as tile
from concourse import bacc, mybir
from concourse.bass_utils import run_bass_kernel_spmd
from concourse.masks import make_identity

B, N, D = 4, 16384, 256
M, H, HD = 64, 8, 32
E = 3 * D  # 768
NSH = N // 2  # 8192 rows per core
P = 128
NT = NSH // P  # 64 tiles of 128 rows
GRP = 4  # tiles per DMA group
NG = NT // GRP  # 16 groups
ATTN_SCALE = 1.0 / math.sqrt(HD)

F32 = mybir.dt.float32
F32R = mybir.dt.float32r

# f32r (1 cyc/row at free>=256) toggles per matmul site; verified vs fp32.
R_LOGITS = False
R_POOL = False
R_V = False
R_WO = False
R_UNPOOL = False

AF = mybir.ActivationFunctionType


def _rr(ap, on=True):
    return ap.bitcast(F32R) if on else ap


def build_program():
    nc = bacc.Bacc(
        "TRN2", target_bir_lowering=False, debug=False, num_devices=8
    )

    x_d = nc.dram_tensor("x", [NSH, D], F32, kind="ExternalInput")
    ws_d = nc.dram_tensor("Ws", [M, D], F32, kind="ExternalInput")
    bs_d = nc.dram_tensor("bs", [M], F32, kind="ExternalInput")
    wqkv_d = nc.dram_tensor("Wqkv", [E, D], F32, kind="ExternalInput")
    wo_d = nc.dram_tensor("Wo", [D, D], F32, kind="ExternalInput")
    bo_d = nc.dram_tensor("bo", [D], F32, kind="ExternalInput")
    out_d = nc.dram_tensor("out", [NSH, D], F32, kind="ExternalOutput")

    xv = x_d.ap().rearrange("(g j p) d -> g p j d", j=GRP, p=P)
    outv = out_d.ap().rearrange("(g j p) d -> g p j d", j=GRP, p=P)

    with tile.TileContext(nc) as tc, ExitStack() as ctx:
        consts = ctx.enter_context(tc.tile_pool(name="consts", bufs=1))
        dram = ctx.enter_context(
            tc.tile_pool(name="dram", bufs=1, space=bass.MemorySpace.DRAM)
        )

        ident = consts.tile([P, P], F32)
        make_identity(nc, ident)

        # ---- weight staging + transposes (one-time) ----
        ws_raw = consts.tile([M, D], F32)
        nc.sync.dma_start(ws_raw[:], ws_d[:])
        wq_raw = consts.tile([P, 6, D], F32)
        nc.sync.dma_start(
            wq_raw[:], wqkv_d.ap().rearrange("(r p) d -> p r d", r=6, p=P)
        )
        wo_raw = consts.tile([P, 2, D], F32)
        nc.sync.dma_start(
            wo_raw[:], wo_d.ap().rearrange("(r p) d -> p r d", r=2, p=P)
        )
        bsT = consts.tile([M, 1], F32)
        nc.sync.dma_start(bsT[:], bs_d.ap().rearrange("(p o) -> p o", o=1))
        bo_b = consts.tile([M, D], F32)
        bo_ap = bo_d.ap()
        nc.sync.dma_start(
            bo_b[:],
            bass.AP(tensor=bo_ap.tensor, offset=bo_ap.offset,
                    ap=[[0, M], list(bo_ap.ap[0])]),
        )

        wsT = consts.tile([P, 2, M], F32)   # [d-in-chunk, c, m]
        wqT = consts.tile([P, 2, E], F32)   # [d-in-chunk, c, e]
        woT = consts.tile([M, 4, D], F32)   # [e'-in-chunk(64), c4, d]
        with tc.tile_pool(name="setup_ps", bufs=2,
                          space=bass.MemorySpace.PSUM) as sps:
            for c in range(2):
                t_ps = sps.tile([P, M], F32, name=f"wsT_ps{c}", tag="w")
                nc.tensor.transpose(t_ps[:], ws_raw[:, c * P:(c + 1) * P],
                                    ident[:M, :M])
                nc.any.tensor_copy(_rr(wsT[:, c, :], R_LOGITS), t_ps[:])
            for r in range(6):
                for c in range(2):
                    t_ps = sps.tile([P, P], F32, name=f"wqT_ps{r}_{c}", tag="w")
                    nc.tensor.transpose(t_ps[:], wq_raw[:, r, c * P:(c + 1) * P],
                                        ident[:])
                    nc.any.tensor_copy(_rr(wqT[:, c, r * P:(r + 1) * P], R_V), t_ps[:])
            for r in range(2):
                for c4 in range(4):
                    t_ps = sps.tile([M, P], F32, name=f"woT_ps{r}_{c4}", tag="w")
                    nc.tensor.transpose(t_ps[:],
                                        wo_raw[:, r, c4 * M:(c4 + 1) * M],
                                        ident[:])
                    nc.any.tensor_copy(_rr(woT[:, c4, r * P:(r + 1) * P], R_WO), t_ps[:])

        # ---- persistent buffers ----
        eT_all = consts.tile([M, NSH], F32)   # exp(logits).T, unnormalized
        tok_sb = consts.tile([M, D + 2], F32)
        tok_full = consts.tile([M, D + 2], F32)
        t_sb = consts.tile([M, D], F32)       # token_out / z, [m, d]
        ar_in = dram.tile([M, D + 2], F32)
        ar_out = dram.tile([M, D + 2], F32)

        # ones source for the 2 appended pooling columns (f32r matmuls
        # need even innermost free counts, and Memset cannot write f32r)
        ones_sb = consts.tile([P, 2 * GRP], F32)
        nc.vector.memset(ones_sb[:], 1.0)
        oap = ones_sb[:]
        ones_src = bass.AP(tensor=oap.tensor, offset=oap.offset,
                           ap=[list(oap.ap[0]), [2, GRP], [1, 2]])

        # ---- pass 1 ----
        with (
            tc.tile_pool(name="xa_pool", bufs=3) as xa_pool,
            tc.tile_pool(name="xT_sb_pool", bufs=2) as xT_sb_pool,
            tc.tile_pool(name="e_sb_pool", bufs=2) as e_sb_pool,
            tc.tile_pool(name="xT_ps_pool", bufs=2,
                         space=bass.MemorySpace.PSUM) as xT_ps_pool,
            tc.tile_pool(name="lg_ps_pool", bufs=2,
                         space=bass.MemorySpace.PSUM) as lg_ps_pool,
            tc.tile_pool(name="e_ps_pool", bufs=2,
                         space=bass.MemorySpace.PSUM) as e_ps_pool,
            tc.tile_pool(name="tok_ps_pool", bufs=1,
                         space=bass.MemorySpace.PSUM) as tok_ps_pool,
        ):
            tok_ps = tok_ps_pool.tile([M, D + 2], F32)
            for g in range(NG):
                xa = xa_pool.tile([P, GRP, D + 2], F32)
                nc.any.tensor_copy(_rr(xa[:, :, D:D + 2], R_POOL), ones_src)
                nc.sync.dma_start(_rr(xa[:, :, 0:D], R_POOL),
                                  _rr(xv[g], R_POOL))
                for pr in range(GRP // 2):  # pairs of tiles
                    i0 = g * GRP + pr * 2          # first tile index of pair
                    # transpose 2 tiles x 2 d-chunks into one PSUM bank:
                    # cols [c*256 + jj*128, +128) = x_tile(i0+jj)[:, cP:cP+P].T
                    xT_ps = xT_ps_pool.tile([P, 512], F32)
                    for c in range(2):
                        for jj in range(2):
                            nc.tensor.transpose(
                                xT_ps[:, c * 256 + jj * P: c * 256 + (jj + 1) * P],
                                xa[:, pr * 2 + jj, c * P:(c + 1) * P],
                                ident[:],
                            )
                    xT_sb = xT_sb_pool.tile([P, 512], F32)
                    nc.any.tensor_copy(_rr(xT_sb[:], R_LOGITS), xT_ps[:])
                    # logitsT [m, n-pair] accumulated over 2 d-chunks
                    lg_ps = lg_ps_pool.tile([M, 256], F32)
                    for c in range(2):
                        nc.tensor.matmul(
                            lg_ps[:],
                            _rr(wsT[:, c, :], R_LOGITS),
                            _rr(xT_sb[:, c * 256:(c + 1) * 256], R_LOGITS),
                            start=(c == 0), stop=(c == 1),
                        )
                    # eT = exp(logitsT + bs) straight into persistent store
                    nc.scalar.activation(
                        _rr(eT_all[:, i0 * P:(i0 + 2) * P], R_UNPOOL),
                        lg_ps[:], AF.Exp, bias=bsT[:], scale=1.0,
                    )
                    # e [n, m] via PE transpose of the eT slices
                    e_ps = e_ps_pool.tile([P, 2 * M], F32)
                    for jj in range(2):
                        nc.tensor.transpose(
                            e_ps[:, jj * M:(jj + 1) * M],
                            eT_all[:, (i0 + jj) * P:(i0 + jj + 1) * P],
                            ident[:M, :M],
                        )
                    e_sb = e_sb_pool.tile([P, 2 * M], F32)
                    nc.any.tensor_copy(_rr(e_sb[:], R_POOL), e_ps[:])
                    # pooled sums [tokens | z] += e.T @ [x | 1]
                    for jj in range(2):
                        it = i0 + jj
                        nc.tensor.matmul(
                            tok_ps[:],
                            _rr(e_sb[:, jj * M:(jj + 1) * M], R_POOL),
                            _rr(xa[:, pr * 2 + jj, :], R_POOL),
                            start=(it == 0), stop=(it == NT - 1),
                        )
            nc.any.tensor_copy(tok_sb[:], tok_ps[:])

        # ---- pair AllReduce of [tokens | z] ----
        nc.gpsimd.dma_start(ar_in[:], tok_sb[:])
        nc.gpsimd.collective_compute(
            "AllReduce",
            mybir.AluOpType.add,
            replica_groups=[[0, 1], [2, 3], [4, 5], [6, 7]],
            ins=[ar_in.opt()],
            outs=[ar_out.opt()],
        )
        nc.gpsimd.dma_start(tok_full[:], ar_out[:])

        # ---- attention over M=64 pooled tokens (replicated) ----
        with (
            tc.tile_pool(name="at_sb", bufs=1) as at_sb,
            tc.tile_pool(name="at_ps", bufs=4,
                         space=bass.MemorySpace.PSUM) as at_ps,
            tc.tile_pool(name="toT_ps_pool", bufs=1,
                         space=bass.MemorySpace.PSUM) as toT_ps_pool,
        ):
            rz = at_sb.tile([M, 1], F32)
            nc.vector.reciprocal(rz[:], tok_full[:, D:D + 1])
            tn = at_sb.tile([M, D], F32)
            nc.vector.tensor_scalar_mul(tn[:], tok_full[:, 0:D], rz[:])

            # tokT [d, m]
            tokT = at_sb.tile([P, 2, M], F32)
            tokT_ps = at_ps.tile([P, 2 * M], F32, tag="at")
            for c in range(2):
                nc.tensor.transpose(tokT_ps[:, c * M:(c + 1) * M],
                                    tn[:, c * P:(c + 1) * P], ident[:M, :M])
            nc.any.tensor_copy(
                _rr(tokT[:].rearrange("p c m -> p (c m)"), R_V), tokT_ps[:]
            )

            # qT, kT [64, 4, m]: chunk j = heads 2j,2j+1 ; v [m, 256]
            qT = at_sb.tile([M, 4, M], F32)
            kT = at_sb.tile([M, 4, M], F32)
            for which, dst in ((0, qT), (1, kT)):
                qk_ps = at_ps.tile([M, 4 * M], F32, name=f"qk_ps{which}", tag="at")
                for j in range(4):
                    off = which * D + j * M
                    for c in range(2):
                        nc.tensor.matmul(
                            qk_ps[:, j * M:(j + 1) * M],
                            wqT[:, c, off:off + M],
                            tokT[:, c, :],
                            start=(c == 0), stop=(c == 1),
                        )
                nc.any.tensor_copy(
                    dst[:].rearrange("p c m -> p (c m)"), qk_ps[:]
                )
            v_ps = at_ps.tile([M, D], F32, tag="at")
            for c in range(2):
                nc.tensor.matmul(
                    v_ps[:],
                    _rr(tokT[:, c, :], R_V),
                    _rr(wqT[:, c, 2 * D:3 * D], R_V),
                    start=(c == 0), stop=(c == 1),
                )
            v_sb = at_sb.tile([M, D], F32)
            nc.any.tensor_copy(v_sb[:], v_ps[:])

            # per-head attention; toT [64, 4, m] (chunk j = heads 2j,2j+1)
            toT_sb = at_sb.tile([M, 4, M], F32)
            toT_ps = toT_ps_pool.tile([M, 4 * M], F32, name="toT_ps")
            for h in range(H):
                chunk = h // 2
                row = (h % 2) * 32
                s_ps = at_ps.tile([M, M], F32, name=f"s_ps{h}", tag="at")
                nc.tensor.matmul(
                    s_ps[:],
                    qT[row:row + 32, chunk, :],
                    kT[row:row + 32, chunk, :],
                )
                mx = at_sb.tile([M, 1], F32, name=f"mx{h}")
                nc.vector.tensor_reduce(
                    mx[:], s_ps[:], axis=mybir.AxisListType.X,
                    op=mybir.AluOpType.max,
                )
                nc.vector.tensor_scalar_mul(mx[:], mx[:], -ATTN_SCALE)
                a_sb = at_sb.tile([M, M], F32, name=f"a_sb{h}")
                nc.scalar.activation(a_sb[:], s_ps[:], AF.Exp,
                                     bias=mx[:], scale=ATTN_SCALE)
                sm = at_sb.tile([M, 1], F32, name=f"sm{h}")
                nc.vector.tensor_reduce(
                    sm[:], a_sb[:], axis=mybir.AxisListType.X,
                    op=mybir.AluOpType.add,
                )
                nc.vector.reciprocal(sm[:], sm[:])
                nc.vector.tensor_scalar_mul(a_sb[:], a_sb[:], sm[:])
                aT_ps = at_ps.tile([M, M], F32, name=f"aT_ps{h}", tag="at")
                nc.tensor.transpose(aT_ps[:], a_sb[:], ident[:M, :M])
                aT_sb = at_sb.tile([M, M], F32, name=f"aT_sb{h}")
                nc.any.tensor_copy(aT_sb[:], aT_ps[:])
                # toT_h [hd, m] = v_h.T @ A.T
                nc.tensor.matmul(
                    toT_ps[row:row + 32, chunk * M:(chunk + 1) * M],
                    v_sb[:, h * 32:(h + 1) * 32],
                    aT_sb[:],
                )
            nc.any.tensor_copy(
                _rr(toT_sb[:].rearrange("p c m -> p (c m)"), R_WO), toT_ps[:]
            )

            # t = (toT.T @ WoT + bo) * rz
            t_ps = at_ps.tile([M, D], F32, name="t_ps", tag="at")
            for c4 in range(4):
                nc.tensor.matmul(
                    t_ps[:],
                    _rr(toT_sb[:, c4, :], R_WO),
                    _rr(woT[:, c4, :], R_WO),
                    start=(c4 == 0), stop=(c4 == 3),
                )
            nc.vector.tensor_add(_rr(t_sb[:], R_UNPOOL), t_ps[:], bo_b[:])
            nc.vector.tensor_scalar_mul(_rr(t_sb[:], R_UNPOOL), t_sb[:], rz[:])

        # ---- pass 2: out = eT.T @ t ----
        with (
            tc.tile_pool(name="o_sb_pool", bufs=3) as o_sb_pool,
            tc.tile_pool(name="o_ps_pool", bufs=4,
                         space=bass.MemorySpace.PSUM) as o_ps_pool,
        ):
            for g in range(NG):
                o_sb = o_sb_pool.tile([P, GRP, D], F32)
                for j in range(GRP):
                    it = g * GRP + j
                    o_ps = o_ps_pool.tile([P, D], F32)
                    nc.tensor.matmul(
                        o_ps[:],
                        _rr(eT_all[:, it * P:(it + 1) * P], R_UNPOOL),
                        _rr(t_sb[:], R_UNPOOL),
                    )
                    nc.any.tensor_copy(o_sb[:, j, :], o_ps[:])
                nc.sync.dma_start(outv[g], o_sb[:])

    nc.compile()
    return nc


_cached = None


def kernel(**inputs: np.ndarray) -> np.ndarray:
    global _cached
    if _cached is None:
        _cached = build_program()
    nc = _cached

    x = np.ascontiguousarray(inputs["x"], dtype=np.float32)
    shared = {
        "Ws": np.ascontiguousarray(inputs["Ws"], dtype=np.float32),
        "bs": np.ascontiguousarray(inputs["bs"], dtype=np.float32),
        "Wqkv": np.ascontiguousarray(inputs["Wqkv"], dtype=np.float32),
        "Wo": np.ascontiguousarray(inputs["Wo"], dtype=np.float32),
        "bo": np.ascontiguousarray(inputs["bo"], dtype=np.float32),
    }
    in_maps = []
    for c in range(8):
        b, half = c // 2, c % 2
        in_maps.append(
            {"x": np.ascontiguousarray(x[b, half * NSH:(half + 1) * NSH, :]),
             **shared}
        )
    res = run_bass_kernel_spmd(nc, in_maps, list(range(8)))
    out = np.empty((B, N, D), dtype=np.float32)
    for c in range(8):
        b, half = c // 2, c % 2
        out[b, half * NSH:(half + 1) * NSH, :] = res.results[c]["out"]
    return out


if __name__ == "__main__":
    rng = np.random.default_rng(0)
    ins = {
        "x": rng.standard_normal((B, N, D), dtype=np.float32),
        "Ws": rng.standard_normal((M, D), dtype=np.float32) / 16,
        "bs": np.zeros((M,), np.float32),
        "Wqkv": rng.standard_normal((E, D), dtype=np.float32) / 16,
        "Wo": rng.standard_normal((D, D), dtype=np.float32) / 16,
        "bo": np.zeros((D,), np.float32),
    }
    o = kernel(**ins)
    print(o.shape, o.dtype, float(np.abs(o).mean()))
